# revision 23
# baseline (speedup 1.0000x reference)
"""CRF negative log-likelihood loss on 8 Trainium2 NeuronCores.

Strategy v3 (data-parallel over batch x host-warmed probe chains):
  - Linear-domain forward recurrence  f' = (M^T f) * exp(em - C)  with
    M = exp(transitions).  The sequence is cut into G=256 segments of
    L=8 steps walked INDEPENDENTLY in parallel.  Each chain's initial
    state is computed ON THE HOST (f64, WH warmup steps, normalized to
    column-sum 1), so the device runs exactly L recurrence slots plus
    one init slot -- no device warmup, no device exp (emissions are
    exp()'d and packed on the host: bf16 for a-units, fp8-e4m3 for
    d-units, re-centered per slot to fit the fp8 normal range).
  - Per slot the [96, 4096] state (two 48-tag chains stacked, blkdiag
    stationary) is processed as 4 independent 1024-col pair-units
    (2 matmuls each; PE moving limit is 512): unit 3 is evacuated+
    multiplied in one DVE scalar_tensor_tensor from PSUM (1x, the STT
    scalar folds the fp8 re-centering); units 0-2 are evacuated by one
    fused ACT Copy (psum->bf16 sbuf) and multiplied by a fused DVE
    tensor_tensor at 2x (all-bf16).  This balances ACT and DVE (the
    two bottleneck engines) while amortizing their per-instruction
    init cost, and 4 independent units hide the MM->evac->mul->MM
    dependency-chain latency.
  - Cross-rep: ex tiles are double-buffered and each rep's tiny cs
    output store is deferred until after the NEXT rep's input DMA
    pieces are queued, so the ex stream is never head-of-line blocked
    (this overlap was worth 10us/rep).
  - One snapshot at the final slot (plus a 32-col early snap for
    chain 0, which is one step shorter): ones-stationary matmuls drop
    per-column sums into the freed psum banks; host takes logs and
    telescopes segment ratios.
  - Gold path score and the combine run on the host in f64.
"""

import numpy as np

B, S, T = 256, 2048, 48
NCORES = 8
BC = B // NCORES            # 32 batch per core
G = 256                     # segments (= chains)
L = S // G                  # owned positions per chain (8)
NSLOT = L + 1               # slots (slot 0 = host-computed init)
WH = 2                      # host warmup steps per chain
STACK = 2                   # vertical stacking (96 = STACK*T rows)
TILEW = G * BC // STACK     # 4096 physical tile columns
MMW = 512                   # cols per matmul (PE moving-dim limit)
UW = 1024                   # cols per evac/mul unit (2 matmuls)
NU = TILEW // UW            # 4 independent pair-units per slot
AUNITS = (0, 1, 2)          # units on the ACT-copy + DVE-2x-mul path
DUNITS = (3,)               # units on the fused DVE-STT path
C_OFF = 4.87                # static per-step log offset
NA = len(AUNITS)
ND = len(DUNITS)
C8 = -0.5                   # fp8 d-stream log centering (slots 1..L-1)
C8L = 2.5                   # fp8 centering for the final slot (ev baked in)
XBF = TILEW + (NSLOT - 1) * NA * UW    # bf16 stream cols (init + a-units)
XF8 = (NSLOT - 1) * ND * UW            # fp8 stream cols (d-units)
CS_W = TILEW + 32           # snapshot cols + chain-0 early-snap cols
# DMA pieces: slot ranges [lo, hi) per piece (slot 0 = init)
PIECES = ((0, 2), (2, 4), (4, 6), (6, 9))
F8PIECES = ((1, 5), (5, 9))


def _bfcol(k, j):
    """bf16-stream column of a-unit j at slot k (k>=1)."""
    return TILEW + (k - 1) * NA * UW + j * UW


def _f8col(k, dj):
    """fp8-stream column of d-unit index dj at slot k (k>=1)."""
    return (k - 1) * ND * UW + dj * UW


def _numpy_crf(emissions, tags, mask, transitions, start_transitions,
               end_transitions):
    """Exact reference (log-space, fp32) — fallback for non-all-ones masks."""
    em = emissions.astype(np.float32)
    tg = tags.astype(np.int64)
    mk = mask.astype(np.int32)
    tr = transitions.astype(np.float32)
    st = start_transitions.astype(np.float32)
    en = end_transitions.astype(np.float32)
    b_idx = np.arange(em.shape[0])
    mf = mk.astype(np.float32)
    gold = st[tg[:, 0]] + em[b_idx, 0, tg[:, 0]]
    trans_sc = tr[tg[:, :-1], tg[:, 1:]]
    emit_sc = np.take_along_axis(em[:, 1:], tg[:, 1:, None], axis=2)[..., 0]
    gold = gold + np.sum((trans_sc + emit_sc) * mf[:, 1:], axis=1)
    last_idx = mk.sum(axis=1) - 1
    gold = gold + en[np.take_along_axis(tg, last_idx[:, None], axis=1)[:, 0]]
    alpha = st[None, :] + em[:, 0]
    for s in range(1, em.shape[1]):
        x = alpha[:, :, None] + tr[None] + em[:, s][:, None, :]
        m = x.max(axis=1)
        nxt = m + np.log(np.exp(x - m[:, None, :]).sum(axis=1))
        alpha = np.where(mk[:, s][:, None] > 0, nxt, alpha)
    x = alpha + en[None, :]
    m = x.max(axis=1)
    fwd = m + np.log(np.exp(x - m[:, None]).sum(axis=1))
    return np.float32(np.mean(fwd - gold))


_CACHE = {}


def _build_module(repeat=1):
    import math
    import concourse.bass as bass
    import concourse.mybir as mybir

    nc = bass.Bass()
    f32 = mybir.dt.float32
    bf16 = mybir.dt.bfloat16
    f8 = mybir.dt.float8e4
    AF = mybir.ActivationFunctionType
    MUL = mybir.AluOpType.mult
    S1 = math.exp(C8 - C_OFF)
    S2 = math.exp(C8L - C_OFF)

    # --- dram params ---
    exd = nc.declare_dram_parameter("exd", [96, XBF], bf16, False)
    exd8 = nc.declare_dram_parameter("exd8", [96, XF8], f8, False)
    mblk = nc.declare_dram_parameter("mblk", [96, 96], bf16, False)
    ones2 = nc.declare_dram_parameter("ones2", [96, 2], bf16, False)
    cs_o = nc.declare_dram_parameter("cs", [2, CS_W], f32, True)

    from contextlib import ExitStack

    with ExitStack() as ctx:
        ec = ctx.enter_context
        m_sb = ec(nc.sbuf_tensor([96, 96], bf16))
        ones2_sb = ec(nc.sbuf_tensor([96, 2], bf16))
        ex_sb = [ec(nc.sbuf_tensor(f"ex_sb{i}", [96, XBF], bf16))
                 for i in range(2)]
        ex8_sb = [ec(nc.sbuf_tensor(f"ex8_sb{i}", [96, XF8], f8))
                  for i in range(2)]
        st_sb = [ec(nc.sbuf_tensor(f"st_sb{i}", [96, TILEW], bf16))
                 for i in range(2)]
        hc_sb = [ec(nc.sbuf_tensor(f"hc_sb{i}", [96, NA * UW], bf16))
                 for i in range(2)]
        cs_sb = ec(nc.sbuf_tensor([2, CS_W], f32))
        pk_ps = ec(nc.psum_tensor("pk_ps", [96, TILEW], f32))
        dma_i = ec(nc.semaphore("dma_i"))
        dma_x = ec(nc.semaphore("dma_x"))
        act_s = ec(nc.semaphore("act_s"))
        pe_s = ec(nc.semaphore("pe_s"))
        dve_s = ec(nc.semaphore("dve_s"))
        dma_o = ec(nc.semaphore("dma_o"))
        block = ec(nc.Block())

        # ---------- planning pass ----------
        plan = {k: [] for k in ("sync", "scalar", "tensor", "vector")}
        cnt = {"dma_i": 0, "dma_x": 0, "act": 0, "pe": 0, "dve": 0,
               "dma_o": 0}
        sems = {"dma_i": dma_i, "dma_x": dma_x, "act": act_s,
                "pe": pe_s, "dve": dve_s, "dma_o": dma_o}

        def emit(eng, waits, fn, inc=None, amount=1):
            plan[eng].append((list(waits), fn, inc, amount))
            if inc is not None:
                cnt[inc] += amount

        # cross-rep state (per pair-unit u)
        piece_done = {}          # (stream, rep, piece) -> dma_x count
        evac_done = [("pe", 0)] * NU      # last evac of unit u (sem, cnt)
        mul_done = [0] * NU               # dve cnt writing st slice u
        tt_done = {}             # (k, u) -> dve cnt of TT (hc WAR)
        prev_out_dma = [0]
        pending_out = [None]     # deferred cs_o store (waits) from prev rep

        def flush_out():
            if pending_out[0] is not None:
                emit("sync", pending_out[0],
                     lambda e: e.dma_start(out=cs_o[:], in_=cs_sb[:]),
                     "dma_o", 16)
                prev_out_dma[0] = cnt["dma_o"]
                pending_out[0] = None
        last_init_reader = {}    # rep -> pe cnt of last MM reading init cols
        last_ex_reader = {}      # rep -> dve cnt of last mul reading ex bufs

        def piece_of_slot(pieces, k):
            for p, (lo, hi) in enumerate(pieces):
                if lo <= k < hi:
                    return p
            raise AssertionError(k)

        def plan_one_rep(rep):
            buf = rep % 2
            exq = ex_sb[buf]
            ex8q = ex8_sb[buf]

            # --- ex DMA pieces (bf16 + fp8 streams interleaved) ---
            def bfrange(lo, hi):
                b0 = 0 if lo == 0 else TILEW + (lo - 1) * NA * UW
                return b0, TILEW + (hi - 1) * NA * UW

            for p, (lo, hi) in enumerate(PIECES):
                waits = []
                if rep >= 2 and p == 0:
                    waits.append(("pe", last_init_reader[rep - 2]))
                    waits.append(("dve", last_ex_reader[rep - 2]))
                c0, c1 = bfrange(lo, hi)
                emit("sync", waits,
                     lambda e, c0=c0, c1=c1, exq=exq: e.dma_start(
                         out=exq[:, c0:c1], in_=exd[:, c0:c1]),
                     "dma_x", 16)
                piece_done[("bf", rep, p)] = cnt["dma_x"]
                if p == 0 and rep == 0:
                    emit("sync", [], lambda e: e.dma_start(out=m_sb[:],
                                                           in_=mblk[:]),
                         "dma_i", 16)
                    emit("sync", [], lambda e: e.dma_start(
                        out=ones2_sb[:], in_=ones2[:]), "dma_i", 16)
                if p < len(F8PIECES):
                    lo8, hi8 = F8PIECES[p]
                    c80, c81 = (lo8 - 1) * ND * UW, (hi8 - 1) * ND * UW
                    emit("sync", [],
                         lambda e, c80=c80, c81=c81, ex8q=ex8q: e.dma_start(
                             out=ex8q[:, c80:c81], in_=exd8[:, c80:c81]),
                         "dma_x", 16)
                    piece_done[("f8", rep, p)] = cnt["dma_x"]
            dmai_done = 32
            # previous rep's output store goes out AFTER this rep's input
            # pieces so it cannot head-of-line-block the ex stream
            flush_out()

            # --- recurrence slots ---
            for k in range(1, NSLOT):
                par = k % 2
                # PE: d-unit matmuls first, then a-units
                mm_of = {}
                for u in list(DUNITS) + list(AUNITS):
                    for h in range(2):
                        i = 2 * u + h
                        wm = [evac_done[u]]
                        if k == 1:
                            wm.append(("dma_x",
                                       piece_done[("bf", rep, 0)]))
                            if rep == 0:
                                wm.append(("dma_i", dmai_done))
                            mov = lambda e, i=i, exq=exq: exq[
                                :, i * MMW:(i + 1) * MMW]
                        else:
                            wm.append(("dve", mul_done[u]))
                            mov = lambda e, i=i, par=par: st_sb[1 - par][
                                :, i * MMW:(i + 1) * MMW]
                        emit("tensor", wm,
                             lambda e, i=i, mov=mov: e.matmul(
                                 pk_ps[:, i * MMW:(i + 1) * MMW],
                                 m_sb[:], mov(e),
                                 start=True, stop=True), "pe", 1)
                        mm_of[i] = cnt["pe"]
                # ACT: fused copies for a-units
                copy_of = {}
                for j, u in enumerate(AUNITS):
                    wc = [("pe", mm_of[2 * u + 1])]
                    if k >= 3:
                        wc.append(("dve", tt_done[(k - 2, u)]))
                    emit("scalar", wc,
                         lambda e, u=u, j=j, par=par: e.activation(
                             hc_sb[par][:, j * UW:(j + 1) * UW],
                             pk_ps[:, u * UW:(u + 1) * UW], AF.Copy),
                         "act", 1)
                    copy_of[u] = cnt["act"]
                # DVE: fused STT for d-units, then 2x TT for a-units
                pd8 = piece_done[("f8", rep, piece_of_slot(F8PIECES, k))]
                pdb = piece_done[("bf", rep, piece_of_slot(PIECES, k))]
                sc = S2 if k == NSLOT - 1 else S1
                for dj, u in enumerate(DUNITS):
                    c0 = _f8col(k, dj)
                    emit("vector", [("pe", mm_of[2 * u + 1]),
                                    ("dma_x", pd8)],
                         lambda e, u=u, par=par, c0=c0, ex8q=ex8q, sc=sc:
                         e.scalar_tensor_tensor(
                             st_sb[par][:, u * UW:(u + 1) * UW],
                             pk_ps[:, u * UW:(u + 1) * UW], sc,
                             ex8q[:, c0:c0 + UW], MUL, MUL), "dve", 1)
                    mul_done[u] = cnt["dve"]
                    evac_done[u] = ("dve", cnt["dve"])
                for j, u in enumerate(AUNITS):
                    c0 = _bfcol(k, j)
                    emit("vector", [("act", copy_of[u]), ("dma_x", pdb)],
                         lambda e, u=u, j=j, par=par, c0=c0, exq=exq:
                         e.tensor_mul(
                             st_sb[par][:, u * UW:(u + 1) * UW],
                             hc_sb[par][:, j * UW:(j + 1) * UW],
                             exq[:, c0:c0 + UW]), "dve", 1)
                    mul_done[u] = cnt["dve"]
                    tt_done[(k, u)] = cnt["dve"]
                    evac_done[u] = ("act", copy_of[u])
                if k == NSLOT - 2:
                    mul15_0 = mul_done[0]
            last_init_reader[rep] = cnt["pe"]
            last_ex_reader[rep] = cnt["dve"]

            # --- snapshot: column sums of st_sb[fpar] (final slot), plus
            #     chain-0 early sums from the slot before; sums land in
            #     rows 0:2 of the psum banks (freed by the final evacs) ---
            fpar = (NSLOT - 1) % 2
            emit("tensor", [("dve", mul15_0), evac_done[0]],
                 lambda e, fpar=fpar: e.matmul(
                     pk_ps[0:2, 0:32], ones2_sb[:],
                     st_sb[1 - fpar][:, 0:32],
                     start=True, stop=True), "pe", 1)
            mini_mm = cnt["pe"]
            emit("scalar", [("pe", mini_mm), ("dma_o", prev_out_dma[0])],
                 lambda e: e.activation(cs_sb[:, TILEW:TILEW + 32],
                                        pk_ps[0:2, 0:32], AF.Copy),
                 "act", 1)
            evac_done[0] = ("act", cnt["act"])
            snap_of = {}
            for i in range(TILEW // MMW):
                u = i // 2
                emit("tensor", [("dve", mul_done[u]), evac_done[u]],
                     lambda e, i=i, fpar=fpar: e.matmul(
                         pk_ps[0:2, i * MMW:(i + 1) * MMW], ones2_sb[:],
                         st_sb[fpar][:, i * MMW:(i + 1) * MMW],
                         start=True, stop=True), "pe", 1)
                snap_of[i] = cnt["pe"]
            # fused snapshot copy-outs: left half on ACT, right on DVE
            half = TILEW // 2
            emit("scalar", [("pe", snap_of[TILEW // MMW // 2 - 1]),
                            ("dma_o", prev_out_dma[0])],
                 lambda e, half=half: e.activation(
                     cs_sb[:, 0:half], pk_ps[0:2, 0:half], AF.Copy),
                 "act", 1)
            for u in range(NU // 2):
                evac_done[u] = ("act", cnt["act"])
            cs_waits = [("act", cnt["act"])]
            emit("vector", [("pe", snap_of[TILEW // MMW - 1]),
                            ("dma_o", prev_out_dma[0])],
                 lambda e, half=half: e.tensor_copy(
                     cs_sb[:, half:TILEW], pk_ps[0:2, half:TILEW]),
                 "dve", 1)
            for u in range(NU // 2, NU):
                evac_done[u] = ("dve", cnt["dve"])
            cs_waits.append(("dve", cnt["dve"]))

            # --- output store (deferred past next rep's input pieces) ---
            pending_out[0] = list(cs_waits)

        for rep in range(repeat):
            plan_one_rep(rep)
        flush_out()
        emit("sync", [("dma_o", cnt["dma_o"])], lambda e: None)

        # ---------- emit into engine streams ----------
        def runner(eng_name):
            def run(engine):
                for waits, fn, _inc, _amt in plan[eng_name]:
                    for sem_name, val in waits:
                        engine.wait_ge(sems[sem_name], val)
                    inst = fn(engine)
                    if _inc is not None and inst is not None:
                        inst.then_inc(sems[_inc], _amt)
            return run

        block.sync(runner("sync"))
        block.scalar(runner("scalar"))
        block.tensor(runner("tensor"))
        block.vector(runner("vector"))

    return nc


def _host_prep(emissions, tags, transitions, start_transitions,
               end_transitions):
    """Per-core input dicts: host-exp'd packed emissions + init states."""
    import ml_dtypes
    bf16 = ml_dtypes.bfloat16
    em = np.ascontiguousarray(emissions, dtype=np.float32)
    tr64 = np.asarray(transitions, dtype=np.float64)
    sv = np.asarray(start_transitions, dtype=np.float64)
    ev = np.asarray(end_transitions, dtype=np.float64)

    M = np.exp(tr64)                                  # [T, T]
    mblk_a = np.zeros((96, 96), np.float64)
    mblk_a[0:T, 0:T] = M
    mblk_a[T:2 * T, T:2 * T] = M
    mblk_a = mblk_a.astype(bf16)
    ones2 = np.zeros((96, 2), bf16)
    ones2[0:T, 0] = 1
    ones2[T:2 * T, 1] = 1

    # ---- init states for ALL cores at once (f64) ----
    # chains g>=1: probe = exp(em[pos0]) normalized; WH-1 mixing steps.
    emf = em.astype(np.float64)                       # [B, S, T]
    gs = np.arange(1, G)
    pos0 = gs * L - WH                                # [G-1]
    a = np.exp(emf[:, pos0])                          # [B, G-1, T]
    a /= a.sum(axis=2, keepdims=True)
    for w in range(1, WH):
        a = np.einsum("ij,bgj->bgi", M.T, a) * np.exp(emf[:, pos0 + w])
        a /= a.sum(axis=2, keepdims=True)
    # chain 0: exact alpha_0 normalized; record log colsum
    a0 = np.exp(sv[None, :] + emf[:, 0])              # [B, T]
    s0 = a0.sum(axis=1)
    a0 = a0 / s0[:, None]
    ln_s0 = np.log(s0)                                # [B]
    init = np.concatenate([a0[:, None, :], a], axis=1)  # [B, G, T]

    from concourse import mybir
    f8np = mybir.dt.np(mybir.dt.float8e4)
    AB = NA * UW
    in_maps = []
    for c in range(NCORES):
        b0 = c * BC
        # ex slots: [NSLOT, T, G*BC] logical; col = g*BC + b
        exv = np.exp(em[b0:b0 + BC] - C_OFF)          # [BC, S, T] f32
        exv[:, S - 1] *= np.exp(ev)[None, :].astype(np.float32)
        slots = np.empty((NSLOT, T, G * BC), np.float32)
        slots[0] = init[b0:b0 + BC].transpose(2, 1, 0).reshape(T, G * BC)
        # slot k uses position g*L + k - 1 for chain g>=1; chain 0's init
        # already includes position 0, so it advances through positions
        # k (its final-slot output is ignored; the early-snap one slot
        # before is its true endpoint)
        posk = (np.arange(G)[None, :] * L
                + np.arange(1, NSLOT)[:, None] - 1)   # [L, G]
        posk[:, 0] = np.arange(1, NSLOT)
        sl = exv[:, posk]                             # [BC, L, G, T]
        slots[1:] = sl.transpose(1, 3, 2, 0).reshape(L, T, G * BC)
        # stack: rows 0:48 = logical cols [0:TILEW), rows 48:96 = rest
        stk = np.concatenate([slots[:, :, 0:TILEW], slots[:, :, TILEW:]],
                             axis=1)                  # [NSLOT, 96, TILEW]
        # bf16 stream: init slot (full width) + a-unit cols per slot
        exbf = np.empty((96, XBF), np.float32)
        exbf[:, 0:TILEW] = stk[0]
        exbf[:, TILEW:] = (stk[1:, :, 0:AB]
                           .transpose(1, 0, 2).reshape(96, L * AB))
        # fp8 stream: d-unit cols, re-centered per slot
        sc8 = np.full(L, np.exp(C_OFF - C8), np.float32)
        sc8[L - 1] = np.exp(C_OFF - C8L)
        e8 = stk[1:, :, AB:TILEW] * sc8[:, None, None]
        e8 = np.clip(e8.transpose(1, 0, 2).reshape(96, XF8), 0, 224)
        in_maps.append({"exd": np.ascontiguousarray(exbf.astype(bf16)),
                        "exd8": np.ascontiguousarray(e8.astype(f8np)),
                        "mblk": mblk_a, "ones2": ones2})
    return in_maps, ln_s0


def _host_gold(emissions, tags, transitions, start_transitions,
               end_transitions):
    """Gold path score per batch (all-ones mask), vectorized float64."""
    em = emissions.astype(np.float64)
    tg = np.asarray(tags).astype(np.int64)
    tr64 = transitions.astype(np.float64)
    b_idx = np.arange(em.shape[0])
    gold = (start_transitions.astype(np.float64)[tg[:, 0]]
            + em[b_idx, 0, tg[:, 0]]
            + tr64[tg[:, :-1], tg[:, 1:]].sum(axis=1)
            + np.take_along_axis(em[:, 1:], tg[:, 1:, None],
                                 axis=2)[..., 0].sum(axis=1)
            + end_transitions.astype(np.float64)[tg[:, -1]])
    return gold


def _ln_s0(emissions, start_transitions):
    """log column-sum of exact alpha_0, per batch element (f64)."""
    em0 = np.asarray(emissions)[:, 0].astype(np.float64)
    sv = np.asarray(start_transitions).astype(np.float64)
    return np.log(np.exp(sv[None, :] + em0).sum(axis=1))


def _combine(results, gold, ln_s0):
    """Host: ln of final column sums, telescoped; minus gold, mean."""
    total = 0.0
    for c, r in enumerate(results):
        cs = r["cs"].astype(np.float64)               # [2, CS_W]
        csg = np.concatenate([cs[0, :TILEW], cs[1, :TILEW]]).reshape(G, BC)
        ln_out = np.log(csg)                          # [G, BC]
        ln_out[0] = np.log(cs[0, TILEW:TILEW + 32])   # chain-0 early snap
        b0 = c * BC
        logz = ln_s0[b0:b0 + BC] + ln_out.sum(axis=0) + (S - 1) * C_OFF
        total += float(np.sum(logz - gold[b0:b0 + BC]))
    return np.float32(total / B)


def kernel(emissions, tags, mask, transitions, start_transitions,
           end_transitions):
    emissions = np.asarray(emissions)
    tags = np.asarray(tags)
    mask = np.asarray(mask)
    transitions = np.asarray(transitions, dtype=np.float32)
    start_transitions = np.asarray(start_transitions, dtype=np.float32)
    end_transitions = np.asarray(end_transitions, dtype=np.float32)

    if not np.all(mask == 1):
        return _numpy_crf(emissions, tags, mask, transitions,
                          start_transitions, end_transitions)

    from concourse.bass_utils import run_bass_kernel_spmd

    if "nc" not in _CACHE:
        _CACHE["nc"] = _build_module()
    nc = _CACHE["nc"]

    in_maps, ln_s0 = _host_prep(emissions, tags, transitions,
                                start_transitions, end_transitions)
    res = run_bass_kernel_spmd(nc, in_maps, core_ids=list(range(NCORES)))
    gold = _host_gold(emissions, tags, transitions, start_transitions,
                      end_transitions)
    return _combine(res.results, gold, ln_s0)


if __name__ == "__main__":
    import jax

    with jax.default_device(jax.devices("cpu")[0]):
        import reference as ref
        inputs = {k: np.asarray(v) for k, v in ref.setup_inputs().items()}
        import jax.numpy as jnp
        expected = float(ref.reference(**{k: jnp.asarray(v)
                                          for k, v in inputs.items()}))
    got = float(kernel(**inputs))
    rel = abs(got - expected) / abs(expected)
    print(f"expected {expected}  got {got}  rel {rel:.3e}")
